# revision 12
# baseline (speedup 1.0000x reference)
"""Distributed Trainium2 kernel for ArticulatoryMetricLoss.

loss = mean_{i != j} ((||e_i||^2 + ||e_j||^2 - 2 e_i.e_j) - art_dist[i, j])^2

Strategy (8 NeuronCores):
  - Shard rows of the pairwise matrix: core c owns rows i in [c*512, (c+1)*512).
    Embeddings are replicated (each core reads the full E^T), so no all-gather
    of embeddings is needed.
  - TRANSPOSED output orientation: each core computes its 4096 x 512 slab
    d2[j, i] with j on partitions (32 j-tiles of 128) and its 512 i's on the
    free dim:
       psum[j, i] = sum_k E^T[k, j] * (-2 E_slab^T)[k, i]   (6 bf16 k-tiles)
                  + 1 * s_i_row[i]                          (K=1 aug matmul,
                                                             local norms only)
  - DVE tensor_tensor_reduce: u = psum - art^T, A2[j] += sum_i u  (one op)
  - ACT: Square(u) with fused accumulation: A1[j] = sum_i u^2
  - The AllGathered norm vector s (needed for the s_j term) enters only at
    the very END via the algebraic expansion
       sum_i (u + s_j)^2 = A1[j] + 2 s_j A2[j] + 512 s_j^2
    so the collective is completely off the critical path (it has ~40us of
    slack while the matmul pipeline runs).
  - All DRAM layouts are per-partition-contiguous ([128, F] with fat rows) so
    every DMA descriptor is >= 2-4KB; row norms are transposed on-chip via a
    small matmul against a host-provided identity (no scatter/gather DMAs).

Numerics: bf16 quantization of E and art gives ~1e-5 relative error on the
final scalar (validated against the fp32 reference in numpy). The diagonal
(i == j) terms are ~0 by construction (consistent quantized norms and gram)
and are simply included; their contribution is ~1e-10 relative.
"""

import os
import sys
from contextlib import ExitStack

import numpy as np

for _p in ("/opt/trn_rl_repo", "/root/.axon_site/_ro/trn_rl_repo"):
    if os.path.isdir(_p) and _p not in sys.path:
        sys.path.insert(0, _p)

import ml_dtypes

import concourse.bass as bass
import concourse.tile as tile
from concourse import bacc, mybir
from concourse.bass_utils import run_bass_kernel_spmd

B = 4096          # rows/cols of the pairwise matrix
D = 768           # embedding dim
NCORES = 8
BP = B // NCORES  # 512 rows per core (free dim of the transposed slab)
P = 128           # SBUF partitions
JT = B // P       # 32 j-tiles per core
KT = D // P       # 6 contraction tiles
MT = BP // P      # 4 row-tiles of the core's slab
PAIRS = B * (B - 1)

BF16 = mybir.dt.bfloat16
F32 = mybir.dt.float32
F8 = mybir.dt.float8e4

_CACHED = {}


def build_graph():
    nc = bacc.Bacc("TRN2", target_bir_lowering=False, debug=False, num_devices=NCORES)

    # per-partition-contiguous packed layouts (fat DMA descriptors)
    lhs_d = nc.dram_tensor("lhs", [P, KT * BP], BF16, kind="ExternalInput")
    rhs_d = nc.dram_tensor("rhs", [P, KT * B], BF16, kind="ExternalInput")
    esl_d = nc.dram_tensor("eslab", [P, MT * D], BF16, kind="ExternalInput")
    art_d = nc.dram_tensor("art", [P, JT * BP], F8, kind="ExternalInput")
    idn_d = nc.dram_tensor("ident", [P, P], F32, kind="ExternalInput")
    out_d = nc.dram_tensor("out", [1, 1], F32, kind="ExternalOutput")

    RHS_CH = 2048   # rhs chunk width (4KB rows)
    N_RHS = KT * B // RHS_CH   # 12
    ART_CH = 2048   # art chunk width (2KB fp8 rows), 4 j-tiles per chunk
    N_ART = JT * BP // ART_CH  # 8

    with tile.TileContext(nc) as tc, ExitStack() as ctx:
        const_pool = ctx.enter_context(tc.tile_pool(name="const", bufs=1))
        rhs_pool = ctx.enter_context(tc.tile_pool(name="rhs", bufs=1))
        lhs_pool = ctx.enter_context(tc.tile_pool(name="lhs", bufs=1))
        art_pool = ctx.enter_context(tc.tile_pool(name="art", bufs=1))
        u_pool = ctx.enter_context(tc.tile_pool(name="u", bufs=8))
        scr_pool = ctx.enter_context(tc.tile_pool(name="scr", bufs=2))
        acc_pool = ctx.enter_context(tc.tile_pool(name="acc", bufs=1))
        psum_pool = ctx.enter_context(tc.tile_pool(name="psum", bufs=7, space="PSUM"))
        psx_pool = ctx.enter_context(tc.tile_pool(name="psx", bufs=1, space="PSUM"))
        dram_pool = ctx.enter_context(tc.tile_pool(name="dram", bufs=1, space="DRAM"))

        # ---- bulk loads.
        # sync (HWDGE): first-needed operands in consumption order - rhs k0h0
        # split fine, lhs, eslab, ident, rest of rhs h0, first h1 halves.
        # gpsimd (SWDGE): art (fp8), late rhs h1 halves, then the AllGather.
        # scalar (HWDGE): the small s-path transfers (so they neither block
        # nor wait on the bulk streams).
        rhs_sub = {}   # (ch, half) -> tile   for the finely-split chunks
        rhs_t = [None] * N_RHS

        def load_rhs(ch, eng):
            rt = rhs_pool.tile([P, RHS_CH], BF16, tag=f"rhs{ch}", name=f"rhs{ch}")
            eng.dma_start(rt[:], rhs_d[:, ch * RHS_CH : (ch + 1) * RHS_CH])
            rhs_t[ch] = rt

        def load_rhs_split(ch, eng):
            # two [128, 1024] sub-chunks so the first batch unblocks sooner
            rt = rhs_pool.tile([P, RHS_CH], BF16, tag=f"rhs{ch}", name=f"rhss{ch}")
            half = RHS_CH // 2
            for h in range(2):
                eng.dma_start(
                    rt[:, h * half : (h + 1) * half],
                    rhs_d[:, ch * RHS_CH + h * half : ch * RHS_CH + (h + 1) * half],
                )
            rhs_t[ch] = rt

        load_rhs_split(0, nc.sync)          # k0 h0 (jt 0..15)
        lhs_t = []
        for ch in range(3):
            lt = lhs_pool.tile([P, KT * BP // 3], BF16, tag=f"lhs{ch}")
            nc.sync.dma_start(
                lt[:], lhs_d[:, ch * (KT * BP // 3) : (ch + 1) * (KT * BP // 3)]
            )
            lhs_t.append(lt)
        esl_t = []
        for ch in range(2):
            et = lhs_pool.tile([P, MT * D // 2], BF16, tag=f"esl{ch}")
            nc.sync.dma_start(
                et[:], esl_d[:, ch * (MT * D // 2) : (ch + 1) * (MT * D // 2)]
            )
            esl_t.append(et)
        ident = const_pool.tile([P, P], F32)
        nc.sync.dma_start(ident[:], idn_d[:])
        for k in range(1, KT):   # remaining h0 halves in k order
            load_rhs(2 * k, nc.sync)
        for k in range(2):       # first two h1 halves on sync as well
            load_rhs(2 * k + 1, nc.sync)

        art_t = []
        for ch in range(N_ART):
            at = art_pool.tile([P, ART_CH], F8, tag=f"art{ch}")
            nc.gpsimd.dma_start(at[:], art_d[:, ch * ART_CH : (ch + 1) * ART_CH])
            art_t.append(at)
        for k in range(2, KT):   # remaining h1 halves (needed from batch ~2.5)
            load_rhs(2 * k + 1, nc.gpsimd)

        def rhs_view(k, jt):  # stationary [128, 128] for (k, jt)
            col = k * B + jt * P
            ch = col // RHS_CH
            off = col % RHS_CH
            return rhs_t[ch][:, off : off + P]

        def lhs_view(k):  # moving [128, 512] for k
            col = k * BP
            ch = col // (KT * BP // 3)
            off = col % (KT * BP // 3)
            return lhs_t[ch][:, off : off + BP]

        def esl_view(m):  # [128, 768] row-block m of the core's slab
            col = m * D
            ch = col // (MT * D // 2)
            off = col % (MT * D // 2)
            return esl_t[ch][:, off : off + D]

        def art_view(jt):  # [128, 512] transposed-art tile jt
            ch = (jt * BP) // ART_CH
            off = (jt * BP) % ART_CH
            return art_t[ch][:, off : off + BP]

        # ---- s path: local row norms -> transpose via identity matmul ->
        # DRAM -> (a) local [1,512] row for aug matmuls (b) AllGather.
        s_sq = acc_pool.tile([P, MT], F32)
        for m in range(MT):
            so = scr_pool.tile([P, D], BF16, tag="scr")
            nc.scalar.activation(
                so[:],
                esl_view(m),
                mybir.ActivationFunctionType.Square,
                accum_out=s_sq[:, m : m + 1],
            )
        psum4 = psx_pool.tile([MT, P], F32, tag="px")
        nc.tensor.matmul(psum4[:], s_sq[:], ident[:], start=True, stop=True)
        sT_bf = const_pool.tile([MT, P], BF16)
        nc.vector.tensor_copy(sT_bf[:], psum4[:])
        s_loc = dram_pool.tile([BP], BF16)
        nc.scalar.dma_start(s_loc[:], sT_bf[:])

        s_i_row = const_pool.tile([1, BP], BF16)
        nc.scalar.dma_start(s_i_row[:], s_loc[:])

        s_all = dram_pool.tile([B], BF16)
        nc.gpsimd.collective_compute(
            "AllGather",
            mybir.AluOpType.bypass,
            replica_groups=[list(range(NCORES))],
            ins=[s_loc[:].opt()],
            outs=[s_all[:].opt()],
        )
        s32 = const_pool.tile([JT, P], BF16)
        identb = const_pool.tile([JT, JT], BF16)
        nc.vector.tensor_copy(identb[:], ident[:JT, :JT])

        ones_lhs = const_pool.tile([1, P], BF16)
        nc.vector.memset(ones_lhs[:], 1.0)
        ones_col = const_pool.tile([P, 1], F32)
        nc.vector.memset(ones_col[:], 1.0)

        # ---- main loop over 32 j-tiles, batches sized to the PSUM pool
        A1 = acc_pool.tile([P, JT], F32)
        A2 = acc_pool.tile([P, JT], F32)
        NBATCH = 7
        for b0 in range(0, JT, NBATCH):
            batch = range(b0, min(b0 + NBATCH, JT))
            psums = {}
            for k in range(KT):
                for jt in batch:
                    if k == 0:
                        psums[jt] = psum_pool.tile([P, BP], F32, tag="ps", name=f"ps{jt}")
                    nc.tensor.matmul(
                        psums[jt][:],
                        rhs_view(k, jt),
                        lhs_view(k),
                        start=(k == 0),
                        stop=False,
                    )
            for jt in batch:
                # += 1 * s_i along the free dim (local norms, no collective)
                nc.tensor.matmul(
                    psums[jt][:], ones_lhs[:], s_i_row[:], start=False, stop=True
                )
            for jt in batch:
                u = u_pool.tile([P, BP], F32, tag="u")
                # u = psum - art ; A2[:, jt] = sum_i(u)   (single DVE op)
                nc.vector.scalar_tensor_tensor(
                    out=u[:],
                    in0=psums[jt][:],
                    scalar=0.0,
                    in1=art_view(jt),
                    op0=mybir.AluOpType.add,
                    op1=mybir.AluOpType.subtract,
                    accum_out=A2[:, jt : jt + 1],
                )
                so = scr_pool.tile([P, D], BF16, tag="scr")
                # A1[:, jt] = sum_i(u^2)
                nc.scalar.activation(
                    so[:, :BP],
                    u[:],
                    mybir.ActivationFunctionType.Square,
                    accum_out=A1[:, jt : jt + 1],
                )

        # ---- s_cols = transpose(s32) via identity matmul: [128, 32] fp32
        nc.scalar.dma_start(s32[:], s_all[:])
        psum32 = psx_pool.tile([P, JT], F32, tag="px")
        nc.tensor.matmul(psum32[:], s32[:], identb[:], start=True, stop=True)
        s_colsF = acc_pool.tile([P, JT], F32)
        nc.vector.tensor_copy(s_colsF[:], psum32[:])

        # ---- combine: T = A1 + 2*s*A2 + 512*s^2 ; reduce all
        t0 = acc_pool.tile([P, JT], F32)
        nc.vector.tensor_scalar_mul(t0[:], A2[:], 2.0)
        t1 = acc_pool.tile([P, JT], F32)
        nc.vector.scalar_tensor_tensor(
            out=t1[:],
            in0=s_colsF[:],
            scalar=float(BP),
            in1=t0[:],
            op0=mybir.AluOpType.mult,
            op1=mybir.AluOpType.add,
        )
        t2 = acc_pool.tile([P, JT], F32)
        nc.vector.tensor_mul(t2[:], t1[:], s_colsF[:])
        t3 = acc_pool.tile([P, JT], F32)
        nc.vector.tensor_add(t3[:], t2[:], A1[:])
        tot = acc_pool.tile([P, 1], F32)
        nc.vector.tensor_reduce(
            tot[:], t3[:], axis=mybir.AxisListType.X, op=mybir.AluOpType.add
        )
        fin = psx_pool.tile([MT, P], F32, tag="px")
        nc.tensor.matmul(fin[0:1, 0:1], tot[:], ones_col[:], start=True, stop=True)
        res = const_pool.tile([1, 1], F32)
        nc.scalar.mul(res[:], fin[0:1, 0:1], 1.0 / PAIRS)
        nc.sync.dma_start(out_d[:], res[:])

    nc.compile()
    return nc


def shard_inputs(embeddings: np.ndarray, art_dist: np.ndarray):
    bf16 = ml_dtypes.bfloat16
    Eb = embeddings.astype(bf16)
    Ebf = Eb.astype(np.float32)

    # rhs[p, k*B + j] = Eb[j, k*128 + p]   (replicated)
    rhs = np.ascontiguousarray(
        Eb.T.reshape(KT, P, B).transpose(1, 0, 2).reshape(P, KT * B)
    )
    ident = np.eye(P, dtype=np.float32)

    in_maps = []
    for c in range(NCORES):
        sl = slice(c * BP, (c + 1) * BP)
        # lhs[p, k*BP + i] = -2 * Eb[c*BP + i, k*128 + p]
        lhs = np.ascontiguousarray(
            (-2.0 * Ebf[sl])
            .astype(bf16)
            .T.reshape(KT, P, BP)
            .transpose(1, 0, 2)
            .reshape(P, KT * BP)
        )
        # esl[p, m*D + d] = Eb[c*BP + m*128 + p, d]
        esl = np.ascontiguousarray(
            Eb[sl].reshape(MT, P, D).transpose(1, 0, 2).reshape(P, MT * D)
        )
        # art[p, jt*BP + f] = A[c*BP + f, jt*128 + p]
        art = np.ascontiguousarray(
            art_dist[sl]
            .T.astype(ml_dtypes.float8_e4m3)
            .reshape(JT, P, BP)
            .transpose(1, 0, 2)
            .reshape(P, JT * BP)
        )
        in_maps.append(
            {"lhs": lhs, "rhs": rhs, "eslab": esl, "art": art, "ident": ident}
        )
    return in_maps


def _get_nc():
    if "nc" not in _CACHED:
        _CACHED["nc"] = build_graph()
    return _CACHED["nc"]


def _ensure_ntff_hook():
    """The agent image's antenv package lacks axon_hooks, so trace=True in
    run_bass_kernel_spmd crashes on import. Recreate the module + register
    the ctypes NTFF hook the way trn_boot would have."""
    try:
        from antenv.axon_hooks import get_axon_ntff_profile_hook  # noqa: F401

        return
    except ImportError:
        pass
    import types

    import antenv

    mod = types.ModuleType("antenv.axon_hooks")
    holder = {"hook": None}
    mod.set_axon_ntff_profile_hook = lambda h: holder.__setitem__("hook", h)
    mod.get_axon_ntff_profile_hook = lambda: holder["hook"]
    sys.modules["antenv.axon_hooks"] = mod
    antenv.axon_hooks = mod
    try:
        from trn_agent_boot.trn_boot import _ntff_profile_via_ctypes

        for so in ("/opt/axon/libaxon_pjrt.so",):
            if os.path.exists(so):
                holder["hook"] = _ntff_profile_via_ctypes(so)
                break
    except Exception as e:  # degrade: tracing skipped, run still works
        print(f"ntff hook setup failed ({e}); tracing disabled", file=sys.stderr)


def run(embeddings: np.ndarray, art_dist: np.ndarray, **run_kwargs):
    if run_kwargs.get("trace"):
        _ensure_ntff_hook()
    nc = _get_nc()
    in_maps = shard_inputs(np.asarray(embeddings), np.asarray(art_dist))
    res = run_bass_kernel_spmd(nc, in_maps, core_ids=list(range(NCORES)), **run_kwargs)
    partials = [np.asarray(r["out"], np.float64).reshape(()) for r in res.results]
    loss = np.float32(np.sum(partials))
    return np.asarray(loss, dtype=np.float32), res


def kernel(embeddings: np.ndarray, art_dist: np.ndarray) -> np.ndarray:
    loss, _ = run(embeddings, art_dist)
    return loss


# revision 15
# speedup vs baseline: 1.4266x; 1.4266x over previous
"""Distributed Trainium2 kernel for ArticulatoryMetricLoss.

loss = mean_{i != j} ((||e_i||^2 + ||e_j||^2 - 2 e_i.e_j) - art_dist[i, j])^2

Strategy (8 NeuronCores), exploiting d2's symmetry (d2_ij == d2_ji):
  - The 8x8 grid of 512x512 (i, j) blocks is covered by its 36 unordered
    block-units (8 diagonal + 28 pairs). Each off-diagonal unit's d2 block is
    computed ONCE and consumed against BOTH art orientations (a_ij and a_ji),
    halving the matmul work. Units are split into 144 [128-j x 512-i]
    sub-jobs, 18 per core (diag unit + gap-1..3 pairs + half of a gap-4
    pair) - every core runs the IDENTICAL graph; which global blocks a core
    works on is decided purely by how the host packs its input buffers.
  - Per sub-job: psum[j, i] = sum_k E^T[k, j] * (-2 E_blk^T)[k, i] over 6
    bf16 k-tiles + a K=1 augmented matmul adding s_i (locally computed
    norms - no collective anywhere).
  - Per pass (1 for diag, 2 for pairs): DVE scalar_tensor_tensor computes
    u = psum - art with a fused free-dim sum (A2), ACT Square computes the
    fused sum of squares (A1).
  - The s_j side of d2 enters only through the algebraic expansion
      sum_i (u + s_j)^2 = A1 + 2 s_j A2 + 512 s_j^2,
    evaluated ON THE HOST from the device outputs (A1, A2, and the
    device-computed bf16 norms) during unsharding. No device collective,
    no tail dependency.

Numerics: bf16 quantization of E, fp8(e4m3) quantization of art, and the
symmetric decomposition give ~1.5e-5 relative error on the final scalar
(validated against the fp32 reference in numpy). Diagonal (i == j) terms
are ~0 by construction (consistent quantized norms and gram) and are simply
included; their contribution is ~1e-10 relative.
"""

import os
import sys
from contextlib import ExitStack

import numpy as np

for _p in ("/opt/trn_rl_repo", "/root/.axon_site/_ro/trn_rl_repo"):
    if os.path.isdir(_p) and _p not in sys.path:
        sys.path.insert(0, _p)

import ml_dtypes

import concourse.bass as bass
import concourse.tile as tile
from concourse import bacc, mybir
from concourse.bass_utils import run_bass_kernel_spmd

B = 4096          # rows/cols of the pairwise matrix
D = 768           # embedding dim
NCORES = 8
BLK = 512         # i/j block size (8x8 block grid)
P = 128           # SBUF partitions
KT = D // P       # 6 contraction tiles
NSUB = 18         # sub-jobs per core
NPASS = 32        # DVE/ACT passes per core
PAIRS = B * (B - 1)

BF16 = mybir.dt.bfloat16
F32 = mybir.dt.float32
F8 = mybir.dt.float8e4

# compile-time per-core structure: (lhs_slot, n_passes) for each sub-job.
# t0..3: diag unit, 1 pass. t4..15: gap-1..3 pairs, 2 passes.
# t16..17: half of the gap-4 pair (aux lhs block), 2 passes.
SUB_STRUCT = [(0, 1)] * 4 + [(0, 2)] * 12 + [(1, 2)] * 2

_CACHED = {}


def subjobs(c):
    """Host-side per-core sub-job table: (bi, bj, jt, npass).

    Must stay in sync with SUB_STRUCT: bi == c for t0..15 (lhs slot 0),
    bi == c % 4 for t16..17 (lhs slot 1)."""
    jobs = []
    for jt in range(4):
        jobs.append((c, c, jt, 1))
    for d in (1, 2, 3):
        for jt in range(4):
            jobs.append((c, (c + d) % 8, jt, 2))
    p = c % 4
    for q in range(2):
        jt = q if c < 4 else q + 2
        jobs.append((p, p + 4, jt, 2))
    return jobs


def build_graph():
    nc = bacc.Bacc("TRN2", target_bir_lowering=False, debug=False, num_devices=NCORES)

    # per-partition-contiguous packed layouts (fat DMA descriptors)
    lhs_d = nc.dram_tensor("lhs", [P, 2 * KT * BLK], BF16, kind="ExternalInput")
    rhs_d = nc.dram_tensor("rhs", [P, NSUB * KT * P], BF16, kind="ExternalInput")
    esl_d = nc.dram_tensor("eslab", [P, 2 * 4 * D], BF16, kind="ExternalInput")
    art_d = nc.dram_tensor("art", [P, NPASS * BLK], F8, kind="ExternalInput")
    idn_d = nc.dram_tensor("ident", [P, P], F32, kind="ExternalInput")
    a1_d = nc.dram_tensor("a1", [P, NPASS], F32, kind="ExternalOutput")
    a2_d = nc.dram_tensor("a2", [P, NPASS], F32, kind="ExternalOutput")
    sn_d = nc.dram_tensor("snorm", [8, P], BF16, kind="ExternalOutput")

    ART_CH = 2048   # art chunk width (2KB fp8 rows), 4 passes per chunk
    N_ART = NPASS * BLK // ART_CH  # 8
    RHS_CH = KT * P  # one sub-job's worth of stationary columns (1.5KB rows)

    with tile.TileContext(nc) as tc, ExitStack() as ctx:
        const_pool = ctx.enter_context(tc.tile_pool(name="const", bufs=1))
        rhs_pool = ctx.enter_context(tc.tile_pool(name="rhs", bufs=1))
        lhs_pool = ctx.enter_context(tc.tile_pool(name="lhs", bufs=1))
        art_pool = ctx.enter_context(tc.tile_pool(name="art", bufs=1))
        u_pool = ctx.enter_context(tc.tile_pool(name="u", bufs=8))
        scr_pool = ctx.enter_context(tc.tile_pool(name="scr", bufs=2))
        acc_pool = ctx.enter_context(tc.tile_pool(name="acc", bufs=1))
        psum_pool = ctx.enter_context(tc.tile_pool(name="psum", bufs=7, space="PSUM"))
        psx_pool = ctx.enter_context(tc.tile_pool(name="psx", bufs=1, space="PSUM"))
        dram_pool = ctx.enter_context(tc.tile_pool(name="dram", bufs=1, space="DRAM"))

        # ---- bulk loads, issue order ~= consumption order.
        # sync (HWDGE): first rhs sub-job chunks, lhs/esl slot 0, ident, rest
        # of rhs. gpsimd (SWDGE): esl slot 1, art (fp8), lhs slot 1.
        rhs_t = [None] * NSUB

        def load_rhs(t, eng):
            rt = rhs_pool.tile([P, RHS_CH], BF16, tag=f"rhs{t}", name=f"rhs{t}")
            eng.dma_start(rt[:], rhs_d[:, t * RHS_CH : (t + 1) * RHS_CH])
            rhs_t[t] = rt

        for t in range(4):
            load_rhs(t, nc.sync)
        lhs_t = []   # six [128, 1024] chunks; chunk ch holds (L, k) = (ch//3, ...)
        for ch in range(3):
            lt = lhs_pool.tile([P, KT * BLK // 3], BF16, tag=f"lhs{ch}", name=f"lhs{ch}")
            nc.sync.dma_start(
                lt[:], lhs_d[:, ch * (KT * BLK // 3) : (ch + 1) * (KT * BLK // 3)]
            )
            lhs_t.append(lt)
        esl_t = []
        for ch in range(2):
            et = lhs_pool.tile([P, 2 * D], BF16, tag=f"esl{ch}", name=f"esl{ch}")
            nc.sync.dma_start(et[:], esl_d[:, ch * 2 * D : (ch + 1) * 2 * D])
            esl_t.append(et)
        ident = const_pool.tile([P, P], F32)
        nc.sync.dma_start(ident[:], idn_d[:])
        for t in range(4, NSUB):
            load_rhs(t, nc.sync)

        for ch in range(2, 4):  # esl slot 1 early on gpsimd (feeds R1 norms)
            et = lhs_pool.tile([P, 2 * D], BF16, tag=f"esl{ch}", name=f"esl{ch}")
            nc.gpsimd.dma_start(et[:], esl_d[:, ch * 2 * D : (ch + 1) * 2 * D])
            esl_t.append(et)
        art_t = []
        for ch in range(N_ART):
            at = art_pool.tile([P, ART_CH], F8, tag=f"art{ch}", name=f"art{ch}")
            nc.gpsimd.dma_start(at[:], art_d[:, ch * ART_CH : (ch + 1) * ART_CH])
            art_t.append(at)
        for ch in range(3, 6):  # lhs slot 1 (only sub-jobs 16-17 need it)
            lt = lhs_pool.tile([P, KT * BLK // 3], BF16, tag=f"lhs{ch}", name=f"lhs{ch}")
            nc.gpsimd.dma_start(
                lt[:], lhs_d[:, ch * (KT * BLK // 3) : (ch + 1) * (KT * BLK // 3)]
            )
            lhs_t.append(lt)

        def rhs_view(t, k):  # stationary [128, 128] for sub-job t, k-tile k
            return rhs_t[t][:, k * P : (k + 1) * P]

        def lhs_view(L, k):  # moving [128, 512] for lhs slot L, k-tile k
            col = (L * KT + k) * BLK
            ch = col // (KT * BLK // 3)
            off = col % (KT * BLK // 3)
            return lhs_t[ch][:, off : off + BLK]

        def esl_view(L, m):  # [128, 768] row-block m of lhs slot L's block
            col = (L * 4 + m) * D
            ch = col // (2 * D)
            off = col % (2 * D)
            return esl_t[ch][:, off : off + D]

        def art_view(pi):  # [128, 512] art tile for pass pi
            ch = (pi * BLK) // ART_CH
            off = (pi * BLK) % ART_CH
            return art_t[ch][:, off : off + BLK]

        # ---- norms: ACT square-accum per row-block, transpose via identity
        # matmul, bounce through DRAM to get bf16 [1, 512] aug rows. All local.
        s_sq = acc_pool.tile([P, 8], F32)
        sT_bfs = []
        s_loc = dram_pool.tile([2 * BLK], BF16)
        s_rows = []
        for L in range(2):
            for m in range(4):
                so = scr_pool.tile([P, D], BF16, tag="scr", name=f"sq{L}{m}")
                nc.scalar.activation(
                    so[:],
                    esl_view(L, m),
                    mybir.ActivationFunctionType.Square,
                    accum_out=s_sq[:, L * 4 + m : L * 4 + m + 1],
                )
            psum4 = psx_pool.tile([4, P], F32, tag="px", name=f"psum4{L}")
            nc.tensor.matmul(
                psum4[:],
                s_sq[:, L * 4 : (L + 1) * 4],
                ident[:],
                start=True,
                stop=True,
            )
            sT_bf = const_pool.tile([4, P], BF16, name=f"sTbf{L}")
            sT_bfs.append(sT_bf)
            nc.vector.tensor_copy(sT_bf[:], psum4[:])
            nc.scalar.dma_start(s_loc[L * BLK : (L + 1) * BLK], sT_bf[:])
            sr = const_pool.tile([1, BLK], BF16, name=f"srow{L}")
            nc.scalar.dma_start(sr[:], s_loc[L * BLK : (L + 1) * BLK])
            s_rows.append(sr)

        ones_lhs = const_pool.tile([1, P], BF16)
        nc.vector.memset(ones_lhs[:], 1.0)

        # ---- main loop over 18 sub-jobs in PSUM-sized batches
        A1 = acc_pool.tile([P, NPASS], F32)
        A2 = acc_pool.tile([P, NPASS], F32)
        pass_of = []  # pass index base per sub-job
        pi = 0
        for t in range(NSUB):
            pass_of.append(pi)
            pi += SUB_STRUCT[t][1]
        assert pi == NPASS

        NBATCH = 7
        for b0 in range(0, NSUB, NBATCH):
            batch = range(b0, min(b0 + NBATCH, NSUB))
            psums = {}
            for k in range(KT):
                for t in batch:
                    if k == 0:
                        psums[t] = psum_pool.tile(
                            [P, BLK], F32, tag="ps", name=f"ps{t}"
                        )
                    nc.tensor.matmul(
                        psums[t][:],
                        rhs_view(t, k),
                        lhs_view(SUB_STRUCT[t][0], k),
                        start=(k == 0),
                        stop=False,
                    )
            for t in batch:
                # += 1 * s_i along the free dim (local norms)
                nc.tensor.matmul(
                    psums[t][:],
                    ones_lhs[:],
                    s_rows[SUB_STRUCT[t][0]][:],
                    start=False,
                    stop=True,
                )
            for t in batch:
                for q in range(SUB_STRUCT[t][1]):
                    p_i = pass_of[t] + q
                    u = u_pool.tile([P, BLK], F32, tag="u", name=f"u{p_i}")
                    # u = psum - art ; A2[:, pi] = sum_i(u)
                    nc.vector.scalar_tensor_tensor(
                        out=u[:],
                        in0=psums[t][:],
                        scalar=0.0,
                        in1=art_view(p_i),
                        op0=mybir.AluOpType.add,
                        op1=mybir.AluOpType.subtract,
                        accum_out=A2[:, p_i : p_i + 1],
                    )
                    so = scr_pool.tile([P, D], BF16, tag="scr", name=f"so{p_i}")
                    # A1[:, pi] = sum_i(u^2)
                    nc.scalar.activation(
                        so[:, :BLK],
                        u[:],
                        mybir.ActivationFunctionType.Square,
                        accum_out=A1[:, p_i : p_i + 1],
                    )

        nc.sync.dma_start(a1_d[:], A1[:])
        nc.sync.dma_start(a2_d[:], A2[:])
        nc.sync.dma_start(sn_d[0:4, :], sT_bfs[0][:])
        nc.sync.dma_start(sn_d[4:8, :], sT_bfs[1][:])

    nc.compile()
    return nc


def shard_inputs(embeddings: np.ndarray, art_dist: np.ndarray):
    bf16 = ml_dtypes.bfloat16
    f8 = ml_dtypes.float8_e4m3
    Eb = embeddings.astype(bf16)
    Ebf = Eb.astype(np.float32)
    ident = np.eye(P, dtype=np.float32)

    def pack_kxf(M, width):  # [rows, width] -> k-tiled [128, (rows//128)*width]
        r = M.shape[0]
        return M.reshape(r // P, P, width).transpose(1, 0, 2).reshape(P, -1)

    in_maps = []
    for c in range(NCORES):
        jobs = subjobs(c)
        lhs_blocks = [c, c % 4]
        # lhs: (-2 E_blk)^T k-tiled, 2 slots
        lhs = np.concatenate(
            [
                pack_kxf((-2.0 * Ebf[b * BLK : (b + 1) * BLK]).astype(bf16).T, BLK)
                for b in lhs_blocks
            ],
            axis=1,
        )
        # eslab: row-major E for the 2 lhs blocks
        esl = np.concatenate(
            [pack_kxf(Eb[b * BLK : (b + 1) * BLK], D) for b in lhs_blocks], axis=1
        )
        # rhs: one [768, 128] k-tiled chunk per sub-job (the j-tile columns)
        rhs = np.concatenate(
            [
                pack_kxf(
                    np.ascontiguousarray(
                        Eb.T[:, bj * BLK + jt * P : bj * BLK + (jt + 1) * P]
                    ),
                    P,
                )
                for (bi, bj, jt, npass) in jobs
            ],
            axis=1,
        )
        # art: one [128 j, 512 i] tile per pass, in pass order
        tiles = []
        for (bi, bj, jt, npass) in jobs:
            i_sl = slice(bi * BLK, (bi + 1) * BLK)
            j_sl = slice(bj * BLK + jt * P, bj * BLK + (jt + 1) * P)
            if npass == 1:
                tiles.append(art_dist[j_sl, i_sl])
            else:
                tiles.append(art_dist[i_sl, j_sl].T)
                tiles.append(art_dist[j_sl, i_sl])
        art = np.concatenate([t.astype(f8) for t in tiles], axis=1)
        in_maps.append(
            {
                "lhs": np.ascontiguousarray(lhs),
                "rhs": np.ascontiguousarray(rhs),
                "eslab": np.ascontiguousarray(esl),
                "art": np.ascontiguousarray(art),
                "ident": ident,
            }
        )
    return in_maps


def combine(results):
    """Host unshard: loss from per-core A1/A2 and device-computed norms."""
    s_glob = np.zeros(B, np.float64)
    for c in range(NCORES):
        s_glob[c * BLK : (c + 1) * BLK] = (
            results[c]["snorm"][0:4].astype(np.float64).reshape(BLK)
        )
    total = 0.0
    for c in range(NCORES):
        A1 = results[c]["a1"].astype(np.float64)
        A2 = results[c]["a2"].astype(np.float64)
        pi = 0
        for (bi, bj, jt, npass) in subjobs(c):
            sj = s_glob[bj * BLK + jt * P : bj * BLK + (jt + 1) * P]
            for q in range(npass):
                total += (A1[:, pi] + 2 * sj * A2[:, pi] + BLK * sj * sj).sum()
                pi += 1
    return np.float32(total / PAIRS)


def _get_nc():
    if "nc" not in _CACHED:
        _CACHED["nc"] = build_graph()
    return _CACHED["nc"]


def _ensure_ntff_hook():
    """The agent image's antenv package lacks axon_hooks, so trace=True in
    run_bass_kernel_spmd crashes on import. Recreate the module + register
    the ctypes NTFF hook the way trn_boot would have."""
    try:
        from antenv.axon_hooks import get_axon_ntff_profile_hook  # noqa: F401

        return
    except ImportError:
        pass
    import types

    import antenv

    mod = types.ModuleType("antenv.axon_hooks")
    holder = {"hook": None}
    mod.set_axon_ntff_profile_hook = lambda h: holder.__setitem__("hook", h)
    mod.get_axon_ntff_profile_hook = lambda: holder["hook"]
    sys.modules["antenv.axon_hooks"] = mod
    antenv.axon_hooks = mod
    try:
        from trn_agent_boot.trn_boot import _ntff_profile_via_ctypes

        for so in ("/opt/axon/libaxon_pjrt.so",):
            if os.path.exists(so):
                holder["hook"] = _ntff_profile_via_ctypes(so)
                break
    except Exception as e:  # degrade: tracing skipped, run still works
        print(f"ntff hook setup failed ({e}); tracing disabled", file=sys.stderr)


def run(embeddings: np.ndarray, art_dist: np.ndarray, **run_kwargs):
    if run_kwargs.get("trace"):
        _ensure_ntff_hook()
    nc = _get_nc()
    in_maps = shard_inputs(np.asarray(embeddings), np.asarray(art_dist))
    res = run_bass_kernel_spmd(nc, in_maps, core_ids=list(range(NCORES)), **run_kwargs)
    loss = combine(res.results)
    return np.asarray(loss, dtype=np.float32), res


def kernel(embeddings: np.ndarray, art_dist: np.ndarray) -> np.ndarray:
    loss, _ = run(embeddings, art_dist)
    return loss
